# revision 7
# baseline (speedup 1.0000x reference)
"""Trainium2 Bass kernel for nn_InterFusion (dense transformer block, S=2).

Strategy:
  - Pure data parallel: batch dim (8192) split across 8 NeuronCores (1024
    rows each); weights replicated.
  - Feature-major on-chip layout: activations live as X^T tiles
    [E-chunk(128) x tokens], so every projection is a natural PE matmul
    chain with no on-chip transposes.  The host pre-transposes the input
    and the weights (free) and re-transposes the output.
  - All matmuls run in float32r (full 1 cycle/row PE rate at N>=256 with
    ~bf16x2 precision).
  - Per-core column (token) layout: col = s*1024 + j, j = row index
    within the core's 1024-row shard (s-major).
  - Four passes, each with its weights SBUF-resident, DRAM spills between:
      A1: LN1 + Q,K,V + attention -> attnout spill
      A2: O-proj + residual(out1 spill) + LN2 (outs spill)
      B (x2): per-position FFN + final residual add.
  - LayerNorm over the joint (2,1024) slice per batch row: PE column-sum
    matmuls (ones lhsT) for sum/sum-of-squares, tiny row math, then PE
    rank-1 broadcast planes (ones-row x m / rstd) consumed by DVE
    scalar_tensor_tensor; the per-feature ln scale/bias is applied with
    tensor_scalar using per-(s,chunk) [128,1] scalars.
  - Attention (seq len 2): elementwise q*k products, per-head reduction
    via a 0/1 head-selector matmul, 2-way softmax, head-expansion via the
    transposed selector matmul, out = p0*(V0-V1) + V1.
"""

import sys

for _p in ("/opt/trn_rl_repo", "/root/.axon_site/_ro/trn_rl_repo"):
    if _p not in sys.path:
        sys.path.append(_p)

import numpy as np

import concourse.bass as bass
import concourse.bacc as bacc
import concourse.tile as tile
from concourse import mybir
from concourse import bass_utils

F32 = mybir.dt.float32
F32R = mybir.dt.float32r
AT = mybir.ActivationFunctionType
OP = mybir.AluOpType

E = 1024
S = 2
B = 8192
NCORES = 8
ROWS = B // NCORES          # 1024 rows per core
TOK = ROWS * S              # 2048 tokens per core
NCH = E // 128              # 8 feature chunks
P = 128
EPS = 1e-5

RA1 = 128                   # A1: 8 tiles of 128 rows (256 tokens)
NT1 = ROWS // RA1
RA2 = 256                   # A2: 4 tiles of 256 rows (512 tokens)
NT2 = ROWS // RA2
TB = 256                    # B: per-position, 4 tiles of 256 tokens
NTB = ROWS // TB

HID = 2 * E
NHID = HID // 128           # 16 hidden chunks


def _ap(handle_ap, offset, dims):
    """Raw AP on the same tensor as handle_ap with explicit offset/dims."""
    return bass.AP(tensor=handle_ap.tensor, offset=handle_ap.offset + offset,
                   ap=[list(d) for d in dims])


def _rep2(tile_ap):
    """Insert a [0,2] replication dim before the last free dim of a 2D AP."""
    pstride, pcount = tile_ap.ap[0]
    fstride, fcount = tile_ap.ap[-1]
    return bass.AP(tensor=tile_ap.tensor, offset=tile_ap.offset,
                   ap=[[pstride, pcount], [0, 2], [fstride, fcount]])


def build(nc):
    # ---------------- DRAM I/O ----------------
    xT = nc.dram_tensor("xT", [E, TOK], F32R, kind="ExternalInput")
    wqT = nc.dram_tensor("wqT", [E, E], F32R, kind="ExternalInput")
    wkT = nc.dram_tensor("wkT", [E, E], F32R, kind="ExternalInput")
    wvT = nc.dram_tensor("wvT", [E, E], F32R, kind="ExternalInput")
    woT = nc.dram_tensor("woT", [E, E], F32R, kind="ExternalInput")
    w11 = nc.dram_tensor("w11", [E, HID], F32R, kind="ExternalInput")
    w12 = nc.dram_tensor("w12", [HID, E], F32R, kind="ExternalInput")
    w21 = nc.dram_tensor("w21", [E, HID], F32R, kind="ExternalInput")
    w22 = nc.dram_tensor("w22", [HID, E], F32R, kind="ExternalInput")
    b11 = nc.dram_tensor("b11", [HID, 1], F32, kind="ExternalInput")
    b12 = nc.dram_tensor("b12", [E, 1], F32, kind="ExternalInput")
    b21 = nc.dram_tensor("b21", [HID, 1], F32, kind="ExternalInput")
    b22 = nc.dram_tensor("b22", [E, 1], F32, kind="ExternalInput")
    # ln scale/bias per position as [E,1] columns
    lw1 = [nc.dram_tensor(f"lw1{s}", [E, 1], F32, kind="ExternalInput")
           for s in range(2)]
    lb1 = [nc.dram_tensor(f"lb1{s}", [E, 1], F32, kind="ExternalInput")
           for s in range(2)]
    lw2 = [nc.dram_tensor(f"lw2{s}", [E, 1], F32, kind="ExternalInput")
           for s in range(2)]
    lb2 = [nc.dram_tensor(f"lb2{s}", [E, 1], F32, kind="ExternalInput")
           for s in range(2)]
    hsel = nc.dram_tensor("hsel", [P, 16], F32R, kind="ExternalInput")
    hselT = nc.dram_tensor("hselT", [16, P], F32R, kind="ExternalInput")
    onesP = nc.dram_tensor("onesP", [P, 1], F32R, kind="ExternalInput")
    onesR = nc.dram_tensor("onesR", [1, P], F32R, kind="ExternalInput")

    outT = nc.dram_tensor("outT", [E, TOK], F32, kind="ExternalOutput")

    # DRAM scratch
    aoT = nc.dram_tensor("aoT", [E, TOK], F32R, kind="Internal")
    o1T = nc.dram_tensor("o1T", [E, TOK], F32R, kind="Internal")
    osT = nc.dram_tensor("osT", [E, TOK], F32R, kind="Internal")

    MM = nc.tensor.matmul

    def dma(out, in_):
        nc.sync.dma_start(out=out, in_=in_)

    def dram2(dten, c, col0, w):
        """[128, 2*w] DRAM AP: feature chunk c, cols col0+[0,w) for s=0
        and 1024+col0+[0,w) for s=1 (s-major token layout)."""
        return _ap(dten.ap(), c * P * TOK + col0,
                   [[TOK, P], [ROWS, 2], [1, w]])

    def ln_consts(pool, wd, bd, pfx):
        """Load per-(s,chunk) [128,1] ln scale/bias tiles."""
        ws, bs = [], []
        for s in range(2):
            wrow, brow = [], []
            for c in range(NCH):
                t = pool.tile([P, 1], F32, tag=f"{pfx}w{s}{c}",
                              name=f"{pfx}w{s}{c}")
                dma(t[:], _ap(wd[s].ap(), c * P, [[1, P], [1, 1]]))
                wrow.append(t)
                t2 = pool.tile([P, 1], F32, tag=f"{pfx}b{s}{c}",
                               name=f"{pfx}b{s}{c}")
                dma(t2[:], _ap(bd[s].ap(), c * P, [[1, P], [1, 1]]))
                brow.append(t2)
            ws.append(wrow)
            bs.append(brow)
        return ws, bs

    def ln_rows(rowp, eps_sb, H, T, sums):
        """Row math from col sums. sums(j) -> (sum_ap_s0, sum_ap_s1) for
        j=0 (x) and j=1 (x^2).  Returns m_full, r_full [1, T] f32r."""
        m = rowp.tile([1, H], F32, tag="m", name="m")
        nc.vector.tensor_add(m[:], *sums(0))
        nc.vector.tensor_scalar_mul(out=m[:], in0=m[:], scalar1=1.0 / (S * E))
        msq = rowp.tile([1, H], F32, tag="msq", name="msq")
        nc.vector.tensor_add(msq[:], *sums(1))
        nc.vector.tensor_scalar_mul(out=msq[:], in0=msq[:],
                                    scalar1=1.0 / (S * E))
        m2 = rowp.tile([1, H], F32, tag="m2", name="m2")
        nc.vector.tensor_mul(m2[:], m[:], m[:])
        var = rowp.tile([1, H], F32, tag="var", name="var")
        nc.vector.tensor_sub(var[:], msq[:], m2[:])
        sd = rowp.tile([1, H], F32, tag="sd", name="sd")
        nc.scalar.activation(out=sd[:], in_=var[:], func=AT.Sqrt,
                             bias=eps_sb[:])
        rstd = rowp.tile([1, H], F32, tag="rstd", name="rstd")
        nc.vector.reciprocal(out=rstd[:], in_=sd[:])
        m_full = rowp.tile([1, T], F32R, tag="m_full", name="m_full")
        nc.vector.tensor_copy(out=m_full[:, 0:H], in_=m[:])
        nc.vector.tensor_copy(out=m_full[:, H:T], in_=m[:])
        r_full = rowp.tile([1, T], F32R, tag="r_full", name="r_full")
        nc.vector.tensor_copy(out=r_full[:, 0:H], in_=rstd[:])
        nc.vector.tensor_copy(out=r_full[:, H:T], in_=rstd[:])
        return m_full, r_full

    def ln_apply(c, src, dst, planes, T, H, lnw, lnb, zpool):
        """dst = ((src - mB) * rB) * w + b  (w,b per s-half)."""
        tmp = zpool.tile([P, T], F32, tag="lntmp", name="lntmp")
        nc.vector.scalar_tensor_tensor(
            out=tmp[:], in0=src[:], scalar=1.0, in1=planes[0],
            op0=OP.mult, op1=OP.subtract)
        z = zpool.tile([P, T], F32, tag="lnz", name="lnz")
        nc.vector.scalar_tensor_tensor(
            out=z[:], in0=tmp[:], scalar=1.0, in1=planes[1],
            op0=OP.mult, op1=OP.mult)
        for s in range(2):
            hs = slice(s * H, (s + 1) * H)
            nc.vector.tensor_scalar(
                out=dst[:, hs], in0=z[:, hs], scalar1=lnw[s][c][:],
                scalar2=lnb[s][c][:], op0=OP.mult, op1=OP.add)

    with tile.TileContext(nc) as tc:
        from contextlib import ExitStack

        # ======================= PASS A1 =======================
        with ExitStack() as ctx:
            wpool = ctx.enter_context(tc.tile_pool(name="a1_w", bufs=1))
            cpool = ctx.enter_context(tc.tile_pool(name="a1_c", bufs=1))
            xp = ctx.enter_context(tc.tile_pool(name="a1_x", bufs=10))
            xsp = ctx.enter_context(tc.tile_pool(name="a1_xs", bufs=8))
            sqp = ctx.enter_context(tc.tile_pool(name="a1_sq", bufs=2))
            qp = ctx.enter_context(tc.tile_pool(name="a1_q", bufs=8))
            kp = ctx.enter_context(tc.tile_pool(name="a1_k", bufs=8))
            vp = ctx.enter_context(tc.tile_pool(name="a1_v", bufs=8))
            prp = ctx.enter_context(tc.tile_pool(name="a1_pr", bufs=3))
            aop = ctx.enter_context(tc.tile_pool(name="a1_ao", bufs=10))
            rowp = ctx.enter_context(tc.tile_pool(name="a1_row", bufs=2))
            smp = ctx.enter_context(tc.tile_pool(name="a1_sm", bufs=2))
            dp = ctx.enter_context(tc.tile_pool(name="a1_d", bufs=3))
            ps_mm = ctx.enter_context(
                tc.tile_pool(name="a1_psmm", bufs=3, space="PSUM"))
            ps_st = ctx.enter_context(
                tc.tile_pool(name="a1_psst", bufs=1, space="PSUM"))
            ps_sb = ctx.enter_context(
                tc.tile_pool(name="a1_pssb", bufs=2, space="PSUM"))
            ps_at = ctx.enter_context(
                tc.tile_pool(name="a1_psat", bufs=1, space="PSUM"))
            ps_px = ctx.enter_context(
                tc.tile_pool(name="a1_pspx", bufs=1, space="PSUM"))

            # resident weights
            wq_sb = [wpool.tile([P, E], F32R, tag=f"wq{c}", name=f"wq{c}")
                     for c in range(NCH)]
            wk_sb = [wpool.tile([P, E], F32R, tag=f"wk{c}", name=f"wk{c}")
                     for c in range(NCH)]
            wv_sb = [wpool.tile([P, E], F32R, tag=f"wv{c}", name=f"wv{c}")
                     for c in range(NCH)]
            for c in range(NCH):
                dma(wq_sb[c][:], _ap(wqT.ap(), c * P * E, [[E, P], [1, E]]))
                dma(wk_sb[c][:], _ap(wkT.ap(), c * P * E, [[E, P], [1, E]]))
                dma(wv_sb[c][:], _ap(wvT.ap(), c * P * E, [[E, P], [1, E]]))
            # constants
            hsel_sb = cpool.tile([P, 16], F32R, tag="hsel", name="hsel_sb")
            hselT_sb = cpool.tile([16, P], F32R, tag="hselT", name="hselT_sb")
            onesP_sb = cpool.tile([P, 1], F32R, tag="onesP", name="onesP_sb")
            onesR_sb = cpool.tile([1, P], F32R, tag="onesR", name="onesR_sb")
            dma(hsel_sb[:], hsel.ap())
            dma(hselT_sb[:], hselT.ap())
            dma(onesP_sb[:], onesP.ap())
            dma(onesR_sb[:], onesR.ap())
            eps_sb = cpool.tile([1, 1], F32, tag="eps", name="eps_sb")
            nc.vector.memset(eps_sb[:], EPS)
            lnw, lnb = ln_consts(cpool, lw1, lb1, "l1")

            H = RA1              # 128 tokens per s-half
            T = 2 * H            # 256 tokens per tile
            for t in range(NT1):
                col0 = t * H
                x_c = [xp.tile([P, T], F32R, tag="x", name="x")
                       for c in range(NCH)]
                for c in range(NCH):
                    dma(x_c[c][:], dram2(xT, c, col0, H))

                # ---- LN1 statistics: col sums of x and x^2 ----
                st = ps_st.tile([1, 2 * T], F32, tag="st", name="st")
                for c in range(NCH):
                    sq = sqp.tile([P, T], F32R, tag="sq", name="sq")
                    nc.scalar.activation(out=sq[:], in_=x_c[c][:],
                                         func=AT.Square)
                    MM(st[:, 0:T], onesP_sb[:], x_c[c][:],
                       start=(c == 0), stop=False)
                    MM(st[:, T:2 * T], onesP_sb[:], sq[:],
                       start=False, stop=(c == NCH - 1))

                stc = rowp.tile([1, 2 * T], F32, tag="stc", name="stc")
                nc.scalar.copy(out=stc[:], in_=st[:])
                m_full, r_full = ln_rows(
                    rowp, eps_sb, H, T,
                    lambda j: (stc[:, j * T:j * T + H],
                               stc[:, j * T + H:(j + 1) * T]))

                # broadcast planes: [mB | rB] in one PSUM bank
                pl = ps_sb.tile([P, 2 * T], F32, tag="pl", name="pl")
                MM(pl[:, 0:T], onesR_sb[:], m_full[:], start=True, stop=False)
                MM(pl[:, T:2 * T], onesR_sb[:], r_full[:],
                   start=False, stop=True)
                planes = (pl[:, 0:T], pl[:, T:2 * T])

                # ---- LN1 apply + Q,K,V ----
                xs_c = []
                for c in range(NCH):
                    xs = xsp.tile([P, T], F32R, tag="xs", name="xs")
                    ln_apply(c, x_c[c], xs, planes, T, H, lnw, lnb, sqp)
                    xs_c.append(xs)

                def proj(w_sb, pool, tag):
                    outs = []
                    for mp in range(NCH // 2):
                        acc = ps_mm.tile([P, 2 * T], F32, tag="acc",
                                         name="acc")
                        for mb in range(2):
                            mcol = (2 * mp + mb) * P
                            for c in range(NCH):
                                MM(acc[:, mb * T:(mb + 1) * T],
                                   w_sb[c][:, mcol:mcol + P], xs_c[c][:],
                                   start=(mb == 0 and c == 0),
                                   stop=(mb == 1 and c == NCH - 1))
                        for mb in range(2):
                            o = pool.tile([P, T], F32, tag=tag, name=tag)
                            nc.scalar.copy(out=o[:],
                                           in_=acc[:, mb * T:(mb + 1) * T])
                            outs.append(o)
                    return outs

                q_c = proj(wq_sb, qp, "q")
                k_c = proj(wk_sb, kp, "k")
                v_c = proj(wv_sb, vp, "v")

                # ---- attention scores ----
                at = ps_at.tile([16, 2 * T], F32, tag="at", name="at")
                for tt in range(2):
                    for c in range(NCH):
                        pr = prp.tile([P, T], F32R, tag="pr", name="pr")
                        nc.vector.tensor_mul(
                            pr[:], q_c[c][:],
                            _rep2(k_c[c][:, tt * H:(tt + 1) * H]))
                        MM(at[:, tt * T:(tt + 1) * T], hsel_sb[:], pr[:],
                           start=(tt == 0 and c == 0),
                           stop=(tt == 1 and c == NCH - 1))
                ats = smp.tile([16, 2 * T], F32, tag="ats", name="ats")
                nc.scalar.copy(out=ats[:], in_=at[:])

                # ---- softmax over t (2 values) ----
                pstage = smp.tile([16, T], F32R, tag="pstage", name="pstage")
                for s in range(2):
                    a0 = ats[:, s * H:(s + 1) * H]
                    a1 = ats[:, T + s * H:T + (s + 1) * H]
                    mx = smp.tile([16, H], F32, tag="mx", name="mx")
                    nc.vector.tensor_max(mx[:], a0, a1)
                    d0 = smp.tile([16, H], F32, tag="d0", name="d0")
                    nc.vector.tensor_sub(d0[:], a0, mx[:])
                    d1 = smp.tile([16, H], F32, tag="d1", name="d1")
                    nc.vector.tensor_sub(d1[:], a1, mx[:])
                    e0 = smp.tile([16, H], F32, tag="e0", name="e0")
                    nc.scalar.activation(out=e0[:], in_=d0[:], func=AT.Exp)
                    e1 = smp.tile([16, H], F32, tag="e1", name="e1")
                    nc.scalar.activation(out=e1[:], in_=d1[:], func=AT.Exp)
                    den = smp.tile([16, H], F32, tag="den", name="den")
                    nc.vector.tensor_add(den[:], e0[:], e1[:])
                    inv = smp.tile([16, H], F32, tag="inv", name="inv")
                    nc.vector.reciprocal(out=inv[:], in_=den[:])
                    nc.vector.tensor_mul(pstage[:, s * H:(s + 1) * H],
                                         e0[:], inv[:])

                # ---- expand p over heads, combine with V ----
                px = ps_px.tile([P, T], F32, tag="px", name="px")
                MM(px[:], hselT_sb[:], pstage[:], start=True, stop=True)
                for c in range(NCH):
                    dd = dp.tile([P, H], F32, tag="dd", name="dd")
                    nc.vector.tensor_sub(dd[:], v_c[c][:, 0:H], v_c[c][:, H:T])
                    ao = aop.tile([P, T], F32R, tag="ao", name="ao")
                    for s in range(2):
                        hs = slice(s * H, (s + 1) * H)
                        tmp2 = dp.tile([P, H], F32, tag="tmp2", name="tmp2")
                        nc.vector.scalar_tensor_tensor(
                            out=tmp2[:], in0=dd[:], scalar=1.0, in1=px[:, hs],
                            op0=OP.mult, op1=OP.mult)
                        nc.vector.scalar_tensor_tensor(
                            out=ao[:, hs], in0=tmp2[:], scalar=1.0,
                            in1=v_c[c][:, H:T], op0=OP.mult, op1=OP.add)
                    dma(dram2(aoT, c, col0, H), ao[:])

        # ======================= PASS A2 =======================
        with ExitStack() as ctx:
            wpool = ctx.enter_context(tc.tile_pool(name="a2_w", bufs=1))
            cpool = ctx.enter_context(tc.tile_pool(name="a2_c", bufs=1))
            xp = ctx.enter_context(tc.tile_pool(name="a2_x", bufs=12))
            aop = ctx.enter_context(tc.tile_pool(name="a2_ao", bufs=12))
            o1p = ctx.enter_context(tc.tile_pool(name="a2_o1", bufs=12))
            osp = ctx.enter_context(tc.tile_pool(name="a2_os", bufs=12))
            sqp = ctx.enter_context(tc.tile_pool(name="a2_sq", bufs=2))
            rowp = ctx.enter_context(tc.tile_pool(name="a2_row", bufs=2))
            ps_mm = ctx.enter_context(
                tc.tile_pool(name="a2_psmm", bufs=3, space="PSUM"))
            ps_st = ctx.enter_context(
                tc.tile_pool(name="a2_psst", bufs=2, space="PSUM"))
            ps_sb = ctx.enter_context(
                tc.tile_pool(name="a2_pssb", bufs=1, space="PSUM"))

            wo_sb = [wpool.tile([P, E], F32R, tag=f"wo{c}", name=f"wo{c}")
                     for c in range(NCH)]
            for c in range(NCH):
                dma(wo_sb[c][:], _ap(woT.ap(), c * P * E, [[E, P], [1, E]]))
            onesP_sb = cpool.tile([P, 1], F32R, tag="onesP", name="onesP_sb")
            onesR_sb = cpool.tile([1, P], F32R, tag="onesR", name="onesR_sb")
            dma(onesP_sb[:], onesP.ap())
            dma(onesR_sb[:], onesR.ap())
            eps_sb = cpool.tile([1, 1], F32, tag="eps", name="eps_sb")
            nc.vector.memset(eps_sb[:], EPS)
            lnw, lnb = ln_consts(cpool, lw2, lb2, "l2")

            H = RA2              # 256
            T = 2 * H            # 512
            for t in range(NT2):
                col0 = t * H
                ao_c = [aop.tile([P, T], F32R, tag="ao", name="ao")
                        for c in range(NCH)]
                x_c = [xp.tile([P, T], F32R, tag="x", name="x")
                       for c in range(NCH)]
                for c in range(NCH):
                    dma(ao_c[c][:], dram2(aoT, c, col0, H))
                    dma(x_c[c][:], dram2(xT, c, col0, H))

                o1_c = []
                for m in range(NCH):
                    acc = ps_mm.tile([P, T], F32, tag="acc", name="acc")
                    for c in range(NCH):
                        MM(acc[:], wo_sb[c][:, m * P:(m + 1) * P], ao_c[c][:],
                           start=(c == 0), stop=(c == NCH - 1))
                    o1 = o1p.tile([P, T], F32R, tag="o1", name="o1")
                    nc.vector.scalar_tensor_tensor(
                        out=o1[:], in0=x_c[m][:], scalar=1.0, in1=acc[:],
                        op0=OP.mult, op1=OP.add)
                    o1_c.append(o1)
                    dma(dram2(o1T, m, col0, H), o1[:])

                # ---- LN2 ----
                sta = ps_st.tile([1, T], F32, tag="st", name="sta")
                stb = ps_st.tile([1, T], F32, tag="st", name="stb")
                for c in range(NCH):
                    sq = sqp.tile([P, T], F32R, tag="sq", name="sq")
                    nc.scalar.activation(out=sq[:], in_=o1_c[c][:],
                                         func=AT.Square)
                    MM(sta[:], onesP_sb[:], o1_c[c][:],
                       start=(c == 0), stop=(c == NCH - 1))
                    MM(stb[:], onesP_sb[:], sq[:],
                       start=(c == 0), stop=(c == NCH - 1))

                stc = rowp.tile([1, 2 * T], F32, tag="stc", name="stc")
                nc.scalar.copy(out=stc[:, 0:T], in_=sta[:])
                nc.scalar.copy(out=stc[:, T:2 * T], in_=stb[:])

                def sums(j):
                    return stc[:, j * T:j * T + H], stc[:, j * T + H:(j + 1) * T]

                m_full, r_full = ln_rows(rowp, eps_sb, H, T, sums)

                # planes: [128, 2T] = 2 banks, separate groups per half
                pl = ps_sb.tile([P, 2 * T], F32, tag="pl", name="pl")
                MM(pl[:, 0:T], onesR_sb[:], m_full[:], start=True, stop=True)
                MM(pl[:, T:2 * T], onesR_sb[:], r_full[:],
                   start=True, stop=True)
                planes = (pl[:, 0:T], pl[:, T:2 * T])

                for c in range(NCH):
                    osv = osp.tile([P, T], F32R, tag="os", name="osv")
                    ln_apply(c, o1_c[c], osv, planes, T, H, lnw, lnb, sqp)
                    dma(dram2(osT, c, col0, H), osv[:])

        # ======================= PASS B (two FFNs) =======================
        for f in range(2):
            w1d, w2d, b1d, b2d = ((w11, w12, b11, b12) if f == 0
                                  else (w21, w22, b21, b22))
            with ExitStack() as ctx:
                wpool = ctx.enter_context(tc.tile_pool(name=f"b{f}_w", bufs=1))
                cpool = ctx.enter_context(tc.tile_pool(name=f"b{f}_c", bufs=1))
                osp = ctx.enter_context(tc.tile_pool(name=f"b{f}_os", bufs=12))
                hp = ctx.enter_context(tc.tile_pool(name=f"b{f}_h", bufs=4))
                o1p = ctx.enter_context(tc.tile_pool(name=f"b{f}_o1", bufs=12))
                yp = ctx.enter_context(tc.tile_pool(name=f"b{f}_y", bufs=4))
                fop = ctx.enter_context(tc.tile_pool(name=f"b{f}_fo", bufs=12))
                ps_h = ctx.enter_context(
                    tc.tile_pool(name=f"b{f}_psh", bufs=3, space="PSUM"))
                ps_y = ctx.enter_context(
                    tc.tile_pool(name=f"b{f}_psy", bufs=4, space="PSUM"))

                w1_sb = [wpool.tile([P, HID], F32R, tag=f"w1{c}",
                                    name=f"w1{c}") for c in range(NCH)]
                for c in range(NCH):
                    dma(w1_sb[c][:],
                        _ap(w1d.ap(), c * P * HID, [[HID, P], [1, HID]]))
                w2_sb = [wpool.tile([P, E], F32R, tag=f"w2{c}",
                                    name=f"w2{c}") for c in range(NHID)]
                for c in range(NHID):
                    dma(w2_sb[c][:],
                        _ap(w2d.ap(), c * P * E, [[E, P], [1, E]]))
                b1_sb = [cpool.tile([P, 1], F32, tag=f"b1{c}", name=f"b1{c}")
                         for c in range(NHID)]
                for c in range(NHID):
                    dma(b1_sb[c][:], _ap(b1d.ap(), c * P, [[1, P], [1, 1]]))
                b2_sb = [cpool.tile([P, 1], F32, tag=f"b2{c}", name=f"b2{c}")
                         for c in range(NCH)]
                for c in range(NCH):
                    dma(b2_sb[c][:], _ap(b2d.ap(), c * P, [[1, P], [1, 1]]))

                T = TB           # 256 tokens (single s)
                for t in range(NTB):
                    col0 = f * ROWS + t * T
                    os_c = [osp.tile([P, T], F32R, tag="os", name="os")
                            for c in range(NCH)]
                    for c in range(NCH):
                        dma(os_c[c][:],
                            _ap(osT.ap(), c * P * TOK + col0,
                                [[TOK, P], [1, T]]))

                    ys = [ps_y.tile([P, 2 * T], F32, tag="ys", name="ys")
                          for _ in range(NCH // 2)]
                    for kc in range(NHID):
                        hacc = ps_h.tile([P, T], F32, tag="hacc", name="hacc")
                        for c in range(NCH):
                            MM(hacc[:], w1_sb[c][:, kc * P:(kc + 1) * P],
                               os_c[c][:], start=(c == 0),
                               stop=(c == NCH - 1))
                        h = hp.tile([P, T], F32R, tag="h", name="h")
                        nc.scalar.activation(out=h[:], in_=hacc[:],
                                             func=AT.Tanh, bias=b1_sb[kc][:])
                        for mp in range(NCH // 2):
                            for mb in range(2):
                                mcol = (2 * mp + mb) * P
                                MM(ys[mp][:, mb * T:(mb + 1) * T],
                                   w2_sb[kc][:, mcol:mcol + P], h[:],
                                   start=(kc == 0 and mb == 0),
                                   stop=(kc == NHID - 1 and mb == 1))

                    for mp in range(NCH // 2):
                        for mb in range(2):
                            m = 2 * mp + mb
                            y = yp.tile([P, T], F32, tag="y", name="y")
                            nc.scalar.activation(
                                out=y[:], in_=ys[mp][:, mb * T:(mb + 1) * T],
                                func=AT.Tanh, bias=b2_sb[m][:])
                            o1 = o1p.tile([P, T], F32, tag="o1", name="o1")
                            dma(o1[:],
                                _ap(o1T.ap(), m * P * TOK + col0,
                                    [[TOK, P], [1, T]]).bitcast(F32))
                            fo = fop.tile([P, T], F32, tag="fo", name="fo")
                            nc.vector.tensor_add(fo[:], y[:], o1[:])
                            dma(_ap(outT.ap(), m * P * TOK + col0,
                                    [[TOK, P], [1, T]]), fo[:])
    return nc


_NC_CACHE = None


def _get_nc():
    global _NC_CACHE
    if _NC_CACHE is None:
        nc = bacc.Bacc("TRN2", target_bir_lowering=False, debug=False)
        build(nc)
        nc.compile()
        _NC_CACHE = nc
    return _NC_CACHE


def _prep_shared(inputs):
    """Host-side weight/constant prep (shared across cores)."""
    f32 = np.float32
    d = {}
    d["wqT"] = np.ascontiguousarray(
        (np.asarray(inputs["Wq"], f32) / 8.0).T)
    d["wkT"] = np.ascontiguousarray(np.asarray(inputs["Wk"], f32).T)
    d["wvT"] = np.ascontiguousarray(np.asarray(inputs["Wv"], f32).T)
    d["woT"] = np.ascontiguousarray(np.asarray(inputs["Wo"], f32).T)
    d["w11"] = np.ascontiguousarray(np.asarray(inputs["f1w1"], f32).T)
    d["w12"] = np.ascontiguousarray(np.asarray(inputs["f1w2"], f32).T)
    d["w21"] = np.ascontiguousarray(np.asarray(inputs["f2w1"], f32).T)
    d["w22"] = np.ascontiguousarray(np.asarray(inputs["f2w2"], f32).T)
    d["b11"] = np.asarray(inputs["f1b1"], f32).reshape(HID, 1)
    d["b12"] = np.asarray(inputs["f1b2"], f32).reshape(E, 1)
    d["b21"] = np.asarray(inputs["f2b1"], f32).reshape(HID, 1)
    d["b22"] = np.asarray(inputs["f2b2"], f32).reshape(E, 1)
    for s in range(2):
        d[f"lw1{s}"] = np.asarray(inputs["ln1_w"], f32)[s].reshape(E, 1).copy()
        d[f"lb1{s}"] = np.asarray(inputs["ln1_b"], f32)[s].reshape(E, 1).copy()
        d[f"lw2{s}"] = np.asarray(inputs["ln2_w"], f32)[s].reshape(E, 1).copy()
        d[f"lb2{s}"] = np.asarray(inputs["ln2_b"], f32)[s].reshape(E, 1).copy()
    hs = np.zeros((P, 16), f32)
    hs[np.arange(P), np.arange(P) % 16] = 1.0
    d["hsel"] = hs
    d["hselT"] = np.ascontiguousarray(hs.T)
    d["onesP"] = np.ones((P, 1), f32)
    d["onesR"] = np.ones((1, P), f32)
    return d


def _prep_core(inputs, core):
    """Per-core transposed input: xT[e, s*1024 + j] = input[rows0+j, s, e]."""
    rows0 = core * ROWS
    chunk = np.asarray(inputs["input"][rows0:rows0 + ROWS], np.float32)
    xT = np.ascontiguousarray(chunk.transpose(2, 1, 0).reshape(E, TOK))
    return xT


def _decode_out(outT):
    """outT [E, s*1024+j] -> [ROWS, S, E]."""
    return np.ascontiguousarray(outT.reshape(E, S, ROWS).transpose(2, 1, 0))


def kernel(**inputs):
    nc = _get_nc()
    shared = _prep_shared(inputs)
    in_maps = []
    for core in range(NCORES):
        m = dict(shared)
        m["xT"] = _prep_core(inputs, core)
        in_maps.append(m)
    res = bass_utils.run_bass_kernel_spmd(nc, in_maps,
                                          core_ids=list(range(NCORES)))
    out = np.empty((B, S, E), np.float32)
    for core in range(NCORES):
        out[core * ROWS:(core + 1) * ROWS] = _decode_out(
            res.results[core]["outT"])
    return out
